# revision 4
# baseline (speedup 1.0000x reference)
"""Single-head causal attention on 8 TRN2 NeuronCores — v4 (hi-pri loads, contig xbar dst).

Problem: x[B=8, T=2048, C=1024], Wq/Wk/Wv[C, H=64] (fp32)
  q = x@Wq; k = x@Wk; v = x@Wv
  wei = softmax(mask(q k^T * C^-0.5)); out = wei @ v       -> [B, T, H]

Sharding: data-parallel over batch, one batch element per core.

Per-core dataflow:
  - x loaded fp32 in 8x 1MB pieces on the scalar HWDGE queue (all queued
    up-front so the HW ring streams them back-to-back at HBM rate),
    cast fp32->bf16 per t-tile on DVE (2x mode), xbar-transposed per
    t-tile on the sync queue.  QKV chunk 0 starts ~9us in, overlapping
    the remaining load.
  - S^T row-packed pairs: even s-blocks at partitions 0:64 (kT copied to
    base 0), odd s-blocks at partitions 64:128 (kT in situ in qk_a +
    duplicated qT at base 64) -> 2 concurrent K=64 matmuls.
  - exp batched per pair [128,1024] across 2 PSUM banks (off-diagonal).
  - causal mask = post-exp affine_select zeroing on bf16 pt (gpsimd).
  - PV accumulates [v|1]^T @ exp(S^T) -> row 64 gives sumexp for free;
    PE-transpose + reciprocal + scale for the final [T,H] output.
"""
import sys

sys.path.insert(0, "/opt/trn_rl_repo")

import numpy as np

import concourse.bass as bass
import concourse.mybir as mybir
import concourse.tile as tile
from concourse import bacc
from concourse.bass_utils import run_bass_kernel_spmd
from concourse.masks import make_identity

B, T, C, H = 8, 2048, 1024, 64
NTT = T // 128   # 16 t-tiles
NCT = C // 128   # 8  c-tiles
NCH = T // 512   # 4  t-chunks
SCALE = float(C) ** -0.5
VP = 80          # v_nat per-tile stride: 160B, 32B-aligned for xbar transpose

F32 = mybir.dt.float32
BF16 = mybir.dt.bfloat16


def build_nc(reps=1):
    nc = bacc.Bacc("TRN2", target_bir_lowering=False, debug=False)
    xD = nc.dram_tensor("x", [T, C], F32, kind="ExternalInput").ap()
    wqD = nc.dram_tensor("Wq", [C, H], F32, kind="ExternalInput").ap()
    wkD = nc.dram_tensor("Wk", [C, H], F32, kind="ExternalInput").ap()
    wvD = nc.dram_tensor("Wv", [C, H], F32, kind="ExternalInput").ap()
    outD = nc.dram_tensor("out", [T, H], F32, kind="ExternalOutput").ap()

    AF = mybir.ActivationFunctionType

    with tile.TileContext(nc) as tc:
        with (
            tc.tile_pool(name="const", bufs=1) as cpool,
            tc.tile_pool(name="xnat", bufs=1) as xnpool,
            tc.tile_pool(name="xt", bufs=1) as xtpool,
            tc.tile_pool(name="qk", bufs=1) as qkpool,
            tc.tile_pool(name="pt", bufs=4) as ptpool,
            tc.tile_pool(name="osb", bufs=3) as opool,
            tc.tile_pool(name="fin", bufs=2) as fpool,
        ):
            # ---- constants ----
            ident = cpool.tile([128, 128], F32)
            make_identity(nc, ident[:])
            wf = cpool.tile([128, NCT, 128], F32)    # [c_lo, c_hi, Wq|Wk] f32
            wvf = cpool.tile([128, NCT, H], F32)
            nc.gpsimd.dma_start(
                wf[:, :, 0:H], wqD.rearrange("(k p) h -> p k h", p=128))
            nc.gpsimd.dma_start(
                wf[:, :, H:128], wkD.rearrange("(k p) h -> p k h", p=128))
            nc.gpsimd.dma_start(
                wvf[:], wvD.rearrange("(k p) h -> p k h", p=128))
            wqk = cpool.tile([128, NCT, 128], BF16)
            wv = cpool.tile([128, NCT, H], BF16)
            nc.vector.tensor_copy(wqk[:], wf[:])
            nc.vector.tensor_copy(wv[:], wvf[:])

            scrap = cpool.tile([128, 1], F32)
            # table preload: first Exp triggers ACT_TABLE_LOAD early
            nc.scalar.activation(scrap[:], ident[:, 0:1], AF.Exp)

            for rep in range(reps):
                emit_body(nc, tc, xD, outD,
                          (wqk, wv, ident),
                          (xnpool, xtpool, qkpool, ptpool, opool, fpool))

    nc.compile()
    return nc


def emit_body(nc, tc, xD, outD, consts, pools):
    AF = mybir.ActivationFunctionType
    ALU = mybir.AluOpType
    wqk, wv, ident = consts
    xnpool, xtpool, qkpool, ptpool, opool, fpool = pools

    x_nat = xnpool.tile([128, NTT, C], F32, tag="xnat")
    x_natb = xnpool.tile([128, NTT, C], BF16, tag="xnatb")
    xt = xtpool.tile([128, NTT, NCT, 128], BF16, tag="xt")
    xR = xD.rearrange("(g p) c -> p g c", p=128)

    qk_a = qkpool.tile([128, T], BF16, tag="qka")   # rows 0:64 qT, 64:128 kT
    kq = qkpool.tile([128, T], BF16, tag="kq")      # rows 0:64 kT, 64:128 qT
    vt = qkpool.tile([64, T], BF16, tag="vt")
    v_nat = qkpool.tile([128, NTT, VP], BF16, tag="vnat")
    nc.gpsimd.memset(v_nat[:, :, H:H + 1], 1.0)
    o_out = fpool.tile([128, NTT, H], F32, tag="oout")
    outR = outD.rearrange("(g p) h -> p g h", p=128)

    def castr(tk):
        # cast piece's t-tile on DVE, then xbar-transpose it on sync
        nc.vector.tensor_copy(x_natb[:, tk, :], x_nat[:, tk, :])
        nc.sync.dma_start(
            xt[:, tk, :, :], x_natb[:, tk, :],
            transpose=True,
        )

    with (
        tc.tile_pool(name="qkps", bufs=1, space="PSUM") as qkps,
        tc.tile_pool(name="aux", bufs=1, space="PSUM") as aux,
        tc.tile_pool(name="ops", bufs=2, space="PSUM") as ops,
        tc.tile_pool(name="stps", bufs=2, space="PSUM") as stps,
    ):
        vps = fps = aux

        def emit_warm(n):
            # PE warm-up gated on the first casted tile; keeps HAM warm
            # through the load lead-in.
            warm = qkps.tile([128, 512], F32, tag="psqk")
            for _ in range(n):
                nc.tensor.matmul(
                    warm[:], x_natb[:, 0, 0:128], x_natb[:, 0, 0:512],
                    start=True, stop=True,
                )

        def emit_qkv(ci):
            sl = slice(ci * 512, (ci + 1) * 512)
            ps_v_t = vps.tile([128, 512], F32, tag="aux")
            ps_v = ps_v_t[0:64, :]
            for k in range(NCT):
                nc.tensor.matmul(
                    ps_v[:], wv[:, k, :], xt[:, ci * 4:(ci + 1) * 4, k, :],
                    start=(k == 0), stop=(k == NCT - 1),
                )
            nc.vector.tensor_copy(vt[:, sl], ps_v[:])
            nc.sync.dma_start(
                v_nat[:, ci * 4:(ci + 1) * 4, 0:H], vt[:, sl], transpose=True
            )
            ps_qk = qkps.tile([128, 512], F32, tag="psqk")
            for k in range(NCT):
                nc.tensor.matmul(
                    ps_qk[:], wqk[:, k, :], xt[:, ci * 4:(ci + 1) * 4, k, :],
                    start=(k == 0), stop=(k == NCT - 1),
                )
            nc.vector.tensor_copy(qk_a[:, sl], ps_qk[:])
            # kq: rows 0:64 <- kT (lhsT for even s-blocks, base 0);
            #     rows 64:128 <- qT duplicate (rhs for odd s-blocks, base 64)
            # SWDGE (gpsimd): SBUF->SBUF concurrent with xbar is safe there.
            nc.gpsimd.dma_start(kq[0:64, sl], qk_a[64:128, sl])
            nc.gpsimd.dma_start(kq[64:128, sl], qk_a[0:64, sl])

        out_pcs = {}

        def emit_attn_core(ci):
            out_pc = ops.tile([H + 1, 512], F32, tag="outc")
            out_pcs[ci] = out_pc
            npair = 2 * ci + 2
            nsb = 4 * ci + 4
            cl, cr = ci * 512, (ci + 1) * 512
            pending = []
            for p in range(npair):
                sbe, sbo = 2 * p, 2 * p + 1
                re, ro = sbe - 4 * ci, sbo - 4 * ci
                t0e, t0o = max(re, 0) * 128, max(ro, 0) * 128
                st = stps.tile([128, 1024], F32, tag="st")
                nc.tensor.matmul(
                    st[:, t0e:512],
                    kq[0:64, sbe * 128:(sbe + 1) * 128],
                    qk_a[0:64, cl + t0e:cr],
                    start=True, stop=True,
                )
                nc.tensor.matmul(
                    st[:, 512 + t0o:1024],
                    qk_a[64:128, sbo * 128:(sbo + 1) * 128],
                    kq[64:128, cl + t0o:cr],
                    start=True, stop=True,
                )
                pt = ptpool.tile([128, 1024], BF16, tag="pt")
                if re < 0:  # fully off-diagonal pair: one batched exp
                    nc.scalar.activation(
                        pt[:, 0:1024], st[:, 0:1024], AF.Exp, scale=SCALE)
                else:
                    nc.scalar.activation(
                        pt[:, t0e:512], st[:, t0e:512], AF.Exp, scale=SCALE)
                    nc.scalar.activation(
                        pt[:, 512 + t0o:1024], st[:, 512 + t0o:1024],
                        AF.Exp, scale=SCALE)
                    # zero upper triangle of the diagonal 128-blocks
                    nc.gpsimd.affine_select(
                        out=pt[:, t0e:t0e + 128], in_=pt[:, t0e:t0e + 128],
                        compare_op=ALU.is_ge, fill=0.0,
                        base=0, pattern=[[1, 128]], channel_multiplier=-1,
                    )
                    nc.gpsimd.affine_select(
                        out=pt[:, 512 + t0o:512 + t0o + 128],
                        in_=pt[:, 512 + t0o:512 + t0o + 128],
                        compare_op=ALU.is_ge, fill=0.0,
                        base=0, pattern=[[1, 128]], channel_multiplier=-1,
                    )
                if pending:
                    for args, kw in pending:
                        nc.tensor.matmul(*args, **kw)
                pending = [
                    ((out_pc[:, t0e:512], v_nat[:, sbe, 0:H + 1],
                      pt[:, t0e:512]),
                     dict(start=(sbe == 0), stop=False)),
                    ((out_pc[:, t0o:512], v_nat[:, sbo, 0:H + 1],
                      pt[:, 512 + t0o:1024]),
                     dict(start=False, stop=(sbo == nsb - 1))),
                ]
            for args, kw in pending:
                nc.tensor.matmul(*args, **kw)

        def emit_attn_out(ci):
            out_pc = out_pcs[ci]
            o_c = opool.tile([H + 1, 512], F32, tag="osb")
            nc.vector.tensor_copy(o_c[:], out_pc[:])
            fin_t = fps.tile([128, 4, 128], F32, tag="aux")
            fin4 = fin_t[:, :, 0:H + 1]
            for rr in range(4):
                nc.tensor.transpose(
                    fin4[:, rr, :],
                    o_c[:, rr * 128:(rr + 1) * 128],
                    ident[0:H + 1, 0:H + 1],
                )
            rcp = fpool.tile([128, 4, 1], F32, tag="rcp")
            nc.vector.reciprocal(rcp[:], fin4[:, :, H:H + 1])
            for rr in range(4):
                tk = ci * 4 + rr
                nc.vector.tensor_scalar_mul(
                    o_out[:, tk, :], fin4[:, rr, 0:H], rcp[:, rr, :]
                )
            nc.gpsimd.dma_start(
                outR[:, ci * 4:(ci + 1) * 4, :],
                o_out[:, ci * 4:(ci + 1) * 4, :],
            )

        # ---- queue all 8 x-piece loads up-front: the scalar HWDGE ring
        # streams them back-to-back at HBM rate, sems fire per-piece ----
        with tc.high_priority():
            for p in range(8):
                nc.scalar.dma_start(x_nat[:, 2 * p:2 * p + 2, :],
                                    xR[:, 2 * p:2 * p + 2, :])

        castr(0)
        castr(1)
        emit_warm(8)
        castr(2)
        castr(3)
        emit_qkv(0)
        castr(4)
        castr(5)
        emit_attn_core(0)
        castr(6)
        castr(7)
        emit_qkv(1)
        emit_attn_out(0)
        castr(8)
        castr(9)
        emit_attn_core(1)
        castr(10)
        castr(11)
        emit_qkv(2)
        emit_attn_out(1)
        castr(12)
        castr(13)
        emit_attn_core(2)
        castr(14)
        castr(15)
        emit_qkv(3)
        emit_attn_out(2)
        emit_attn_core(3)
        emit_attn_out(3)


_NC = None


def kernel(x, Wq, Wk, Wv):
    global _NC
    if _NC is None:
        _NC = build_nc()
    in_maps = [
        {
            "x": np.ascontiguousarray(x[b], dtype=np.float32),
            "Wq": np.ascontiguousarray(Wq, dtype=np.float32),
            "Wk": np.ascontiguousarray(Wk, dtype=np.float32),
            "Wv": np.ascontiguousarray(Wv, dtype=np.float32),
        }
        for b in range(B)
    ]
    res = run_bass_kernel_spmd(_NC, in_maps, core_ids=list(range(B)))
    return np.stack([res.results[b]["out"] for b in range(B)], axis=0)


# revision 5
# speedup vs baseline: 1.0596x; 1.0596x over previous
"""Single-head causal attention on 8 TRN2 NeuronCores — v5 (SWDGE loads, copy-free QKV packing).

Problem: x[B=8, T=2048, C=1024], Wq/Wk/Wv[C, H=64] (fp32)
  q = x@Wq; k = x@Wk; v = x@Wv
  wei = softmax(mask(q k^T * C^-0.5)); out = wei @ v       -> [B, T, H]

Sharding: data-parallel over batch, one batch element per core.

Per-core dataflow:
  - x loaded fp32 in 8x 1MB pieces on the gpsimd SWDGE queue.  SWDGE has
    its own completion-semaphore pool, so the loads never share lanes
    with the transposes (the HWDGE lane pool round-robins across queues
    and convoys the pipeline otherwise).  Cast fp32->bf16 per t-tile on
    DVE (2x mode) / ScalarE, xbar-transpose per t-tile on sync.
  - Projections packed so every S operand lands where it's needed with
    zero SBUF->SBUF copies:
      [Wk]    -> ka:  kT at partitions 0:64
      [Wq|Wv] -> qv:  qT at partitions 0:64, v at 64:128
    S^T block = ka_block.T @ qv[0:64]  (K=64, tile(0,0))
  - exp batched per block-pair [128,1024] across 2 PSUM banks.
  - causal mask = post-exp affine_select zeroing on bf16 pt (gpsimd).
  - PV accumulates [v|1]^T @ exp(S^T) -> row 64 gives sumexp for free;
    PE-transpose + reciprocal + scale for the final [T,H] output.
"""
import sys

sys.path.insert(0, "/opt/trn_rl_repo")

import numpy as np

import concourse.bass as bass
import concourse.mybir as mybir
import concourse.tile as tile
from concourse import bacc
from concourse.bass_utils import run_bass_kernel_spmd
from concourse.masks import make_identity

B, T, C, H = 8, 2048, 1024, 64
NTT = T // 128   # 16 t-tiles
NCT = C // 128   # 8  c-tiles
NCH = T // 512   # 4  t-chunks
SCALE = float(C) ** -0.5
VP = 80          # v_nat per-tile stride: 160B, 32B-aligned for xbar transpose

F32 = mybir.dt.float32
BF16 = mybir.dt.bfloat16


def build_nc(reps=1):
    nc = bacc.Bacc("TRN2", target_bir_lowering=False, debug=False)
    xD = nc.dram_tensor("x", [T, C], F32, kind="ExternalInput").ap()
    wqD = nc.dram_tensor("Wq", [C, H], F32, kind="ExternalInput").ap()
    wkD = nc.dram_tensor("Wk", [C, H], F32, kind="ExternalInput").ap()
    wvD = nc.dram_tensor("Wv", [C, H], F32, kind="ExternalInput").ap()
    outD = nc.dram_tensor("out", [T, H], F32, kind="ExternalOutput").ap()

    AF = mybir.ActivationFunctionType

    with tile.TileContext(nc) as tc:
        with (
            tc.tile_pool(name="const", bufs=1) as cpool,
            tc.tile_pool(name="xnat", bufs=1) as xnpool,
            tc.tile_pool(name="xt", bufs=1) as xtpool,
            tc.tile_pool(name="qk", bufs=1) as qkpool,
            tc.tile_pool(name="pt", bufs=4) as ptpool,
            tc.tile_pool(name="osb", bufs=3) as opool,
            tc.tile_pool(name="fin", bufs=2) as fpool,
        ):
            # ---- constants ----
            ident = cpool.tile([128, 128], F32)
            make_identity(nc, ident[:])
            # W loads ride the scalar HWDGE ring (front, one-shot); the
            # SWDGE ring stays exclusively for the 8 x-piece loads.
            wf = cpool.tile([128, NCT, 128], F32)    # [c_lo, c_hi, Wq|Wv] f32
            wkf = cpool.tile([128, NCT, H], F32)
            nc.scalar.dma_start(
                wf[:, :, 0:H], wqD.rearrange("(k p) h -> p k h", p=128))
            nc.scalar.dma_start(
                wf[:, :, H:128], wvD.rearrange("(k p) h -> p k h", p=128))
            nc.scalar.dma_start(
                wkf[:], wkD.rearrange("(k p) h -> p k h", p=128))
            wqv = cpool.tile([128, NCT, 128], BF16)
            wk = cpool.tile([128, NCT, H], BF16)
            nc.vector.tensor_copy(wqv[:], wf[:])
            nc.vector.tensor_copy(wk[:], wkf[:])

            scrap = cpool.tile([128, 1], F32)
            # table preload: first Exp triggers ACT_TABLE_LOAD early
            nc.scalar.activation(scrap[:], ident[:, 0:1], AF.Exp)

            for rep in range(reps):
                emit_body(nc, tc, xD, outD,
                          (wqv, wk, ident),
                          (xnpool, xtpool, qkpool, ptpool, opool, fpool))

    nc.compile()
    return nc


def emit_body(nc, tc, xD, outD, consts, pools):
    AF = mybir.ActivationFunctionType
    ALU = mybir.AluOpType
    wqv, wk, ident = consts
    xnpool, xtpool, qkpool, ptpool, opool, fpool = pools

    x_nat = xnpool.tile([128, NTT, C], F32, tag="xnat")
    x_natb = xnpool.tile([128, NTT, C], BF16, tag="xnatb")
    xt = xtpool.tile([128, NTT, NCT, 128], BF16, tag="xt")
    xR = xD.rearrange("(g p) c -> p g c", p=128)

    qv = qkpool.tile([128, T], BF16, tag="qv")    # rows 0:64 qT, 64:128 v
    ka = qkpool.tile([64, T], BF16, tag="ka")     # kT at partitions 0:64
    v_nat = qkpool.tile([128, NTT, VP], BF16, tag="vnat")
    nc.gpsimd.memset(v_nat[:, :, H:H + 1], 1.0)
    o_out = fpool.tile([128, NTT, H], F32, tag="oout")
    outR = outD.rearrange("(g p) h -> p g h", p=128)

    ACT_CAST = (1, 3)   # these tiles cast on ScalarE (early, ACT is idle)

    def castr(tk):
        if tk in ACT_CAST:
            nc.scalar.copy(x_natb[:, tk, :], x_nat[:, tk, :])
        else:
            nc.vector.tensor_copy(x_natb[:, tk, :], x_nat[:, tk, :])
        nc.sync.dma_start(
            xt[:, tk, :, :], x_natb[:, tk, :],
            transpose=True,
        )

    with (
        tc.tile_pool(name="qkps", bufs=1, space="PSUM") as qkps,
        tc.tile_pool(name="aux", bufs=1, space="PSUM") as aux,
        tc.tile_pool(name="ops", bufs=2, space="PSUM") as ops,
        tc.tile_pool(name="stps", bufs=2, space="PSUM") as stps,
    ):
        vps = fps = aux

        def emit_warm(n):
            # PE warm-up gated on the first casted tile; keeps HAM warm
            # through the load lead-in.
            warm = qkps.tile([128, 512], F32, tag="psqk")
            for _ in range(n):
                nc.tensor.matmul(
                    warm[:], x_natb[:, 0, 0:128], x_natb[:, 0, 0:512],
                    start=True, stop=True,
                )

        def emit_qkv(ci):
            sl = slice(ci * 512, (ci + 1) * 512)
            ps_qv = vps.tile([128, 512], F32, tag="aux")
            for k in range(NCT):
                nc.tensor.matmul(
                    ps_qv[:], wqv[:, k, :], xt[:, ci * 4:(ci + 1) * 4, k, :],
                    start=(k == 0), stop=(k == NCT - 1),
                )
            nc.vector.tensor_copy(qv[:, sl], ps_qv[:])
            nc.sync.dma_start(
                v_nat[:, ci * 4:(ci + 1) * 4, 0:H], qv[64:128, sl],
                transpose=True,
            )
            ps_k_t = qkps.tile([128, 512], F32, tag="psqk")
            ps_k = ps_k_t[0:64, :]
            for k in range(NCT):
                nc.tensor.matmul(
                    ps_k[:], wk[:, k, :], xt[:, ci * 4:(ci + 1) * 4, k, :],
                    start=(k == 0), stop=(k == NCT - 1),
                )
            nc.vector.tensor_copy(ka[:, sl], ps_k[:])

        out_pcs = {}

        def emit_attn_core(ci):
            out_pc = ops.tile([H + 1, 512], F32, tag="outc")
            out_pcs[ci] = out_pc
            npair = 2 * ci + 2
            nsb = 4 * ci + 4
            cl, cr = ci * 512, (ci + 1) * 512
            pending = []
            for p in range(npair):
                sbe, sbo = 2 * p, 2 * p + 1
                re, ro = sbe - 4 * ci, sbo - 4 * ci
                t0e, t0o = max(re, 0) * 128, max(ro, 0) * 128
                st = stps.tile([128, 1024], F32, tag="st")
                nc.tensor.matmul(
                    st[:, t0e:512],
                    ka[0:64, sbe * 128:(sbe + 1) * 128],
                    qv[0:64, cl + t0e:cr],
                    start=True, stop=True,
                )
                nc.tensor.matmul(
                    st[:, 512 + t0o:1024],
                    ka[0:64, sbo * 128:(sbo + 1) * 128],
                    qv[0:64, cl + t0o:cr],
                    start=True, stop=True,
                )
                pt = ptpool.tile([128, 1024], BF16, tag="pt")
                if re < 0:  # fully off-diagonal pair: one batched exp
                    nc.scalar.activation(
                        pt[:, 0:1024], st[:, 0:1024], AF.Exp, scale=SCALE)
                else:
                    nc.scalar.activation(
                        pt[:, t0e:512], st[:, t0e:512], AF.Exp, scale=SCALE)
                    nc.scalar.activation(
                        pt[:, 512 + t0o:1024], st[:, 512 + t0o:1024],
                        AF.Exp, scale=SCALE)
                    # zero upper triangle of the diagonal 128-blocks
                    nc.gpsimd.affine_select(
                        out=pt[:, t0e:t0e + 128], in_=pt[:, t0e:t0e + 128],
                        compare_op=ALU.is_ge, fill=0.0,
                        base=0, pattern=[[1, 128]], channel_multiplier=-1,
                    )
                    nc.gpsimd.affine_select(
                        out=pt[:, 512 + t0o:512 + t0o + 128],
                        in_=pt[:, 512 + t0o:512 + t0o + 128],
                        compare_op=ALU.is_ge, fill=0.0,
                        base=0, pattern=[[1, 128]], channel_multiplier=-1,
                    )
                if pending:
                    for args, kw in pending:
                        nc.tensor.matmul(*args, **kw)
                pending = [
                    ((out_pc[:, t0e:512], v_nat[:, sbe, 0:H + 1],
                      pt[:, t0e:512]),
                     dict(start=(sbe == 0), stop=False)),
                    ((out_pc[:, t0o:512], v_nat[:, sbo, 0:H + 1],
                      pt[:, 512 + t0o:1024]),
                     dict(start=False, stop=(sbo == nsb - 1))),
                ]
            for args, kw in pending:
                nc.tensor.matmul(*args, **kw)

        def emit_attn_out(ci):
            out_pc = out_pcs[ci]
            o_c = opool.tile([H + 1, 512], F32, tag="osb")
            nc.vector.tensor_copy(o_c[:], out_pc[:])
            fin_t = fps.tile([128, 4, 128], F32, tag="aux")
            fin4 = fin_t[:, :, 0:H + 1]
            for rr in range(4):
                nc.tensor.transpose(
                    fin4[:, rr, :],
                    o_c[:, rr * 128:(rr + 1) * 128],
                    ident[0:H + 1, 0:H + 1],
                )
            rcp = fpool.tile([128, 4, 1], F32, tag="rcp")
            nc.vector.reciprocal(rcp[:], fin4[:, :, H:H + 1])
            for rr in range(4):
                tk = ci * 4 + rr
                nc.vector.tensor_scalar_mul(
                    o_out[:, tk, :], fin4[:, rr, 0:H], rcp[:, rr, :]
                )
            nc.scalar.dma_start(
                outR[:, ci * 4:(ci + 1) * 4, :],
                o_out[:, ci * 4:(ci + 1) * 4, :],
            )

        # ---- all 8 x-piece loads queued up-front on the SWDGE ring:
        # it has nothing else on it, so they stream at HBM rate and their
        # completion sems come from the SW pool (no HWDGE lane sharing).
        for p in range(8):
            nc.gpsimd.dma_start(x_nat[:, 2 * p:2 * p + 2, :],
                                xR[:, 2 * p:2 * p + 2, :])

        castr(0)
        castr(1)
        emit_warm(6)
        castr(2)
        castr(3)
        emit_qkv(0)
        castr(4)
        castr(5)
        emit_attn_core(0)
        castr(6)
        castr(7)
        emit_qkv(1)
        emit_attn_out(0)
        castr(8)
        castr(9)
        emit_attn_core(1)
        castr(10)
        castr(11)
        emit_qkv(2)
        emit_attn_out(1)
        castr(12)
        castr(13)
        emit_attn_core(2)
        castr(14)
        castr(15)
        emit_qkv(3)
        emit_attn_out(2)
        emit_attn_core(3)
        emit_attn_out(3)


_NC = None


def kernel(x, Wq, Wk, Wv):
    global _NC
    if _NC is None:
        _NC = build_nc()
    in_maps = [
        {
            "x": np.ascontiguousarray(x[b], dtype=np.float32),
            "Wq": np.ascontiguousarray(Wq, dtype=np.float32),
            "Wk": np.ascontiguousarray(Wk, dtype=np.float32),
            "Wv": np.ascontiguousarray(Wv, dtype=np.float32),
        }
        for b in range(B)
    ]
    res = run_bass_kernel_spmd(_NC, in_maps, core_ids=list(range(B)))
    return np.stack([res.results[b]["out"] for b in range(B)], axis=0)


# revision 6
# speedup vs baseline: 1.2938x; 1.2210x over previous
"""Single-head causal attention on 8 TRN2 NeuronCores — v6 (chunk-granular load/cast/transpose).

Problem: x[B=8, T=2048, C=1024], Wq/Wk/Wv[C, H=64] (fp32)
  q = x@Wq; k = x@Wk; v = x@Wv
  wei = softmax(mask(q k^T * C^-0.5)); out = wei @ v       -> [B, T, H]

Sharding: data-parallel over batch, one batch element per core.

Per-core dataflow:
  - x loaded fp32 in 8x 1MB pieces on the gpsimd SWDGE queue.  SWDGE has
    its own completion-semaphore pool, so the loads never share lanes
    with the transposes (the HWDGE lane pool round-robins across queues
    and convoys the pipeline otherwise).  Cast fp32->bf16 per t-tile on
    DVE (2x mode) / ScalarE, xbar-transpose per t-tile on sync.
  - Projections packed so every S operand lands where it's needed with
    zero SBUF->SBUF copies:
      [Wk]    -> ka:  kT at partitions 0:64
      [Wq|Wv] -> qv:  qT at partitions 0:64, v at 64:128
    S^T block = ka_block.T @ qv[0:64]  (K=64, tile(0,0))
  - exp batched per block-pair [128,1024] across 2 PSUM banks.
  - causal mask = post-exp affine_select zeroing on bf16 pt (gpsimd).
  - PV accumulates [v|1]^T @ exp(S^T) -> row 64 gives sumexp for free;
    PE-transpose + reciprocal + scale for the final [T,H] output.
"""
import sys

sys.path.insert(0, "/opt/trn_rl_repo")

import numpy as np

import concourse.bass as bass
import concourse.mybir as mybir
import concourse.tile as tile
from concourse import bacc
from concourse.bass_utils import run_bass_kernel_spmd
from concourse.masks import make_identity

B, T, C, H = 8, 2048, 1024, 64
NTT = T // 128   # 16 t-tiles
NCT = C // 128   # 8  c-tiles
NCH = T // 512   # 4  t-chunks
SCALE = float(C) ** -0.5
VP = 80          # v_nat per-tile stride: 160B, 32B-aligned for xbar transpose

F32 = mybir.dt.float32
BF16 = mybir.dt.bfloat16


def build_nc(reps=1):
    nc = bacc.Bacc("TRN2", target_bir_lowering=False, debug=False)
    xD = nc.dram_tensor("x", [T, C], F32, kind="ExternalInput").ap()
    wqD = nc.dram_tensor("Wq", [C, H], F32, kind="ExternalInput").ap()
    wkD = nc.dram_tensor("Wk", [C, H], F32, kind="ExternalInput").ap()
    wvD = nc.dram_tensor("Wv", [C, H], F32, kind="ExternalInput").ap()
    outD = nc.dram_tensor("out", [T, H], F32, kind="ExternalOutput").ap()

    AF = mybir.ActivationFunctionType

    with tile.TileContext(nc) as tc:
        with (
            tc.tile_pool(name="const", bufs=1) as cpool,
            tc.tile_pool(name="xnat", bufs=1) as xnpool,
            tc.tile_pool(name="xt", bufs=1) as xtpool,
            tc.tile_pool(name="qk", bufs=1) as qkpool,
            tc.tile_pool(name="pt", bufs=4) as ptpool,
            tc.tile_pool(name="osb", bufs=3) as opool,
            tc.tile_pool(name="fin", bufs=2) as fpool,
        ):
            # ---- constants ----
            ident = cpool.tile([128, 128], F32)
            make_identity(nc, ident[:])
            # W loads ride the scalar HWDGE ring (front, one-shot); the
            # SWDGE ring stays exclusively for the 8 x-piece loads.
            wf = cpool.tile([128, NCT, 128], F32)    # [c_lo, c_hi, Wq|Wv] f32
            wkf = cpool.tile([128, NCT, H], F32)
            nc.scalar.dma_start(
                wf[:, :, 0:H], wqD.rearrange("(k p) h -> p k h", p=128))
            nc.scalar.dma_start(
                wf[:, :, H:128], wvD.rearrange("(k p) h -> p k h", p=128))
            nc.scalar.dma_start(
                wkf[:], wkD.rearrange("(k p) h -> p k h", p=128))
            wqv = cpool.tile([128, NCT, 128], BF16)
            wk = cpool.tile([128, NCT, H], BF16)
            nc.vector.tensor_copy(wqv[:], wf[:])
            nc.vector.tensor_copy(wk[:], wkf[:])

            scrap = cpool.tile([128, 1], F32)
            # table preload: first Exp triggers ACT_TABLE_LOAD early
            nc.scalar.activation(scrap[:], ident[:, 0:1], AF.Exp)

            for rep in range(reps):
                emit_body(nc, tc, xD, outD,
                          (wqv, wk, ident),
                          (xnpool, xtpool, qkpool, ptpool, opool, fpool))

    nc.compile()
    return nc


def emit_body(nc, tc, xD, outD, consts, pools):
    AF = mybir.ActivationFunctionType
    ALU = mybir.AluOpType
    wqv, wk, ident = consts
    xnpool, xtpool, qkpool, ptpool, opool, fpool = pools

    x_nat = xnpool.tile([128, NTT, C], F32, tag="xnat")
    x_natb = xnpool.tile([128, NTT, C], BF16, tag="xnatb")
    xt = xtpool.tile([128, NTT, NCT, 128], BF16, tag="xt")
    xR = xD.rearrange("(g p) c -> p g c", p=128)

    qv = qkpool.tile([128, T], BF16, tag="qv")    # rows 0:64 qT, 64:128 v
    ka = qkpool.tile([64, T], BF16, tag="ka")     # kT at partitions 0:64
    v_nat = qkpool.tile([128, NTT, VP], BF16, tag="vnat")
    nc.gpsimd.memset(v_nat[:, :, H:H + 1], 1.0)
    o_out = fpool.tile([128, NTT, H], F32, tag="oout")
    outR = outD.rearrange("(g p) h -> p g h", p=128)

    def castc(ci):
        # cast + transpose a whole 512-t chunk (4 t-tiles) in one op each:
        # all t-tiles share the same 128 partitions, so one xbar transpose
        # of [128, 4096] lands each 128-col group in its own (tile, c_grp)
        # slot of xt.  Fewer, bigger DMA ops -> no descriptor-slot convoy.
        nc.vector.tensor_copy(x_natb[:, 4 * ci:4 * ci + 4, :],
                              x_nat[:, 4 * ci:4 * ci + 4, :])
        nc.sync.dma_start(
            xt[:, 4 * ci:4 * ci + 4, :, :], x_natb[:, 4 * ci:4 * ci + 4, :],
            transpose=True,
        )

    with (
        tc.tile_pool(name="qkps", bufs=1, space="PSUM") as qkps,
        tc.tile_pool(name="aux", bufs=1, space="PSUM") as aux,
        tc.tile_pool(name="ops", bufs=2, space="PSUM") as ops,
        tc.tile_pool(name="stps", bufs=2, space="PSUM") as stps,
    ):
        vps = fps = aux

        def emit_warm(n):
            # PE warm-up gated on the weight cast (ready ~3us); keeps HAM
            # warm through the load lead-in until QKV(0).
            warm = qkps.tile([128, 512], F32, tag="psqk")
            for _ in range(n):
                nc.tensor.matmul(
                    warm[:], wqv[:, 0, :], wqv[:, 0:4, :].opt(),
                    start=True, stop=True,
                )

        def emit_qkv(ci):
            sl = slice(ci * 512, (ci + 1) * 512)
            ps_qv = vps.tile([128, 512], F32, tag="aux")
            for k in range(NCT):
                nc.tensor.matmul(
                    ps_qv[:], wqv[:, k, :], xt[:, ci * 4:(ci + 1) * 4, k, :],
                    start=(k == 0), stop=(k == NCT - 1),
                )
            nc.vector.tensor_copy(qv[:, sl], ps_qv[:])
            nc.sync.dma_start(
                v_nat[:, ci * 4:(ci + 1) * 4, 0:H], qv[64:128, sl],
                transpose=True,
            )
            ps_k_t = qkps.tile([128, 512], F32, tag="psqk")
            ps_k = ps_k_t[0:64, :]
            for k in range(NCT):
                nc.tensor.matmul(
                    ps_k[:], wk[:, k, :], xt[:, ci * 4:(ci + 1) * 4, k, :],
                    start=(k == 0), stop=(k == NCT - 1),
                )
            nc.vector.tensor_copy(ka[:, sl], ps_k[:])

        out_pcs = {}

        def emit_attn_core(ci):
            out_pc = ops.tile([H + 1, 512], F32, tag="outc")
            out_pcs[ci] = out_pc
            npair = 2 * ci + 2
            nsb = 4 * ci + 4
            cl, cr = ci * 512, (ci + 1) * 512
            pending = []
            for p in range(npair):
                sbe, sbo = 2 * p, 2 * p + 1
                re, ro = sbe - 4 * ci, sbo - 4 * ci
                t0e, t0o = max(re, 0) * 128, max(ro, 0) * 128
                st = stps.tile([128, 1024], F32, tag="st")
                nc.tensor.matmul(
                    st[:, t0e:512],
                    ka[0:64, sbe * 128:(sbe + 1) * 128],
                    qv[0:64, cl + t0e:cr],
                    start=True, stop=True,
                )
                nc.tensor.matmul(
                    st[:, 512 + t0o:1024],
                    ka[0:64, sbo * 128:(sbo + 1) * 128],
                    qv[0:64, cl + t0o:cr],
                    start=True, stop=True,
                )
                pt = ptpool.tile([128, 1024], BF16, tag="pt")
                if re < 0:  # fully off-diagonal pair: one batched exp
                    nc.scalar.activation(
                        pt[:, 0:1024], st[:, 0:1024], AF.Exp, scale=SCALE)
                else:
                    nc.scalar.activation(
                        pt[:, t0e:512], st[:, t0e:512], AF.Exp, scale=SCALE)
                    nc.scalar.activation(
                        pt[:, 512 + t0o:1024], st[:, 512 + t0o:1024],
                        AF.Exp, scale=SCALE)
                    # zero upper triangle of the diagonal 128-blocks
                    nc.gpsimd.affine_select(
                        out=pt[:, t0e:t0e + 128], in_=pt[:, t0e:t0e + 128],
                        compare_op=ALU.is_ge, fill=0.0,
                        base=0, pattern=[[1, 128]], channel_multiplier=-1,
                    )
                    nc.gpsimd.affine_select(
                        out=pt[:, 512 + t0o:512 + t0o + 128],
                        in_=pt[:, 512 + t0o:512 + t0o + 128],
                        compare_op=ALU.is_ge, fill=0.0,
                        base=0, pattern=[[1, 128]], channel_multiplier=-1,
                    )
                if pending:
                    for args, kw in pending:
                        nc.tensor.matmul(*args, **kw)
                pending = [
                    ((out_pc[:, t0e:512], v_nat[:, sbe, 0:H + 1],
                      pt[:, t0e:512]),
                     dict(start=(sbe == 0), stop=False)),
                    ((out_pc[:, t0o:512], v_nat[:, sbo, 0:H + 1],
                      pt[:, 512 + t0o:1024]),
                     dict(start=False, stop=(sbo == nsb - 1))),
                ]
            for args, kw in pending:
                nc.tensor.matmul(*args, **kw)

        def emit_attn_out(ci):
            out_pc = out_pcs[ci]
            o_c = opool.tile([H + 1, 512], F32, tag="osb")
            nc.vector.tensor_copy(o_c[:], out_pc[:])
            fin_t = fps.tile([128, 4, 128], F32, tag="aux")
            fin4 = fin_t[:, :, 0:H + 1]
            for rr in range(4):
                nc.tensor.transpose(
                    fin4[:, rr, :],
                    o_c[:, rr * 128:(rr + 1) * 128],
                    ident[0:H + 1, 0:H + 1],
                )
            rcp = fpool.tile([128, 4, 1], F32, tag="rcp")
            nc.vector.reciprocal(rcp[:], fin4[:, :, H:H + 1])
            nc.vector.tensor_tensor(
                o_out[:, ci * 4:(ci + 1) * 4, :], fin4[:, :, 0:H],
                rcp[:].to_broadcast([128, 4, H]), op=ALU.mult,
            )
            nc.scalar.dma_start(
                outR[:, ci * 4:(ci + 1) * 4, :],
                o_out[:, ci * 4:(ci + 1) * 4, :],
            )

        # ---- all 8 x-piece loads queued up-front on the SWDGE ring:
        # it has nothing else on it, so they stream at HBM rate and their
        # completion sems come from the SW pool (no HWDGE lane sharing).
        for p in range(4):
            nc.gpsimd.dma_start(x_nat[:, 4 * p:4 * p + 4, :],
                                xR[:, 4 * p:4 * p + 4, :])

        emit_warm(16)
        castc(0)
        emit_qkv(0)
        castc(1)
        emit_attn_core(0)
        emit_qkv(1)
        castc(2)
        emit_attn_out(0)
        emit_attn_core(1)
        emit_qkv(2)
        castc(3)
        emit_attn_out(1)
        emit_attn_core(2)
        emit_qkv(3)
        emit_attn_out(2)
        emit_attn_core(3)
        emit_attn_out(3)


_NC = None


def kernel(x, Wq, Wk, Wv):
    global _NC
    if _NC is None:
        _NC = build_nc()
    in_maps = [
        {
            "x": np.ascontiguousarray(x[b], dtype=np.float32),
            "Wq": np.ascontiguousarray(Wq, dtype=np.float32),
            "Wk": np.ascontiguousarray(Wk, dtype=np.float32),
            "Wv": np.ascontiguousarray(Wv, dtype=np.float32),
        }
        for b in range(B)
    ]
    res = run_bass_kernel_spmd(_NC, in_maps, core_ids=list(range(B)))
    return np.stack([res.results[b]["out"] for b in range(B)], axis=0)
